# revision 8
# baseline (speedup 1.0000x reference)
"""Trainium2 Bass kernel: MultiHeadDepthwiseSelfAttention.

Full inputs -> data-parallel over batch across 8 NeuronCores -> full output.

Math (reference):
    q = x*wq + bq; k = x*wk + bk; v = x*wv + bv      (per-channel depthwise)
    att = softmax(q @ k^T / sqrt(F))  per head        (H=12, d=64)
    out = (att @ v) * wo + bo

Kernel strategy (per core, 2 batches):
  - Host folds the output projection into v:  veff = wv*wo, beff = bv*wo + bo.
    Then out = (att @ (x*veff + beff_aug)) / Z  where the +bo fold works because
    appending bo[c] to every V row adds bo[c]*Z[q] to the unnormalized output.
  - On chip, work in the transposed orientation S^T[k,q] so softmax's
    normalization sum (over k) is computed by the PV matmul itself via an
    extra ones-column appended per head to V ("Vhat": H*(d+1) columns).
    exp() needs no max-subtraction: logits are O(1) by construction.
  - x^T is produced by PE transposes; Q^T/K^T by per-partition scale+bias
    (channels live on partitions in transposed space).
  - Matmuls run as float32r (TF32-like, 1 cycle/row at N>=256; ~1e-4 rel err).
  - Unnormalized O^T[d+1, q] is PE-transposed back to natural layout where the
    Z row becomes a column: per-partition reciprocal * mul finishes softmax.
"""

import math
import os
import sys

for _p in ("/opt/trn_rl_repo", "/root/.axon_site/_ro/trn_rl_repo"):
    if os.path.isdir(_p) and _p not in sys.path:
        sys.path.insert(0, _p)

import numpy as np

import concourse.bacc as bacc
import concourse.mybir as mybir
from concourse.tile import TileContext
from concourse.masks import make_identity
from concourse.bass_utils import run_bass_kernel_spmd

FP32 = mybir.dt.float32
FP32R = mybir.dt.float32r
AF = mybir.ActivationFunctionType
ALU = mybir.AluOpType

P = 128
N_CORES = 8
B, N, F, H = 16, 1024, 768, 12


def build(BPC=2, N=N, F=F, H=H):
    d = F // H            # head dim (64)
    dO = d + 1            # V columns per head incl. ones column
    NT = N // P           # n-tiles (= k-chunks)
    CT = F // P           # channel chunks
    QB = min(512, N)      # q block (moving-dim) size
    QC = N // QB          # q blocks
    KG = 2                # k-chunks per exp group (psum_s tile spans KG banks)
    NG = NT // KG         # exp groups per (h, qblock)
    TB = QB // P          # natural q-subtiles per q block
    scale = 1.0 / math.sqrt(F)
    assert P % d == 0

    nc = bacc.Bacc("TRN2", target_bir_lowering=False, debug=False,
                   num_devices=N_CORES)
    x = nc.declare_dram_parameter("x", [BPC, N, F], FP32, isOutput=False)
    wq = nc.declare_dram_parameter("wq", [F], FP32, isOutput=False)
    bq = nc.declare_dram_parameter("bq", [F], FP32, isOutput=False)
    wk = nc.declare_dram_parameter("wk", [F], FP32, isOutput=False)
    bk = nc.declare_dram_parameter("bk", [F], FP32, isOutput=False)
    veff = nc.declare_dram_parameter("veff", [F], FP32, isOutput=False)
    beff = nc.declare_dram_parameter("beff", [F], FP32, isOutput=False)
    out = nc.declare_dram_parameter("out", [BPC, N, F], FP32, isOutput=True)

    with TileContext(nc) as tc:
        with (
            tc.tile_pool(name="const", bufs=1) as cpool,
            tc.tile_pool(name="xp", bufs=1) as xpool,
            tc.tile_pool(name="qp", bufs=1) as qpool,
            tc.tile_pool(name="kp", bufs=1) as kpool,
            tc.tile_pool(name="vp", bufs=1) as vpool,
            tc.tile_pool(name="op", bufs=1) as opool,
            tc.tile_pool(name="ptp", bufs=3) as ptpool,
            tc.tile_pool(name="otp", bufs=2) as otpool,
            tc.tile_pool(name="rzp", bufs=2) as rzpool,
            tc.tile_pool(name="ps_s", bufs=2, space="PSUM") as ps_s,
            tc.tile_pool(name="ps_o", bufs=2, space="PSUM") as ps_o,
            tc.tile_pool(name="ps_t", bufs=2, space="PSUM") as ps_t,
        ):
            ident = cpool.tile([P, P], FP32)
            make_identity(nc, ident[:])

            # per-partition weight columns: [P, CT], column c = channels c*128..
            wq_c = cpool.tile([P, CT], FP32)
            bq_c = cpool.tile([P, CT], FP32)
            wk_c = cpool.tile([P, CT], FP32)
            bk_c = cpool.tile([P, CT], FP32)
            for t, src in ((wq_c, wq), (bq_c, bq), (wk_c, wk), (bk_c, bk)):
                nc.sync.dma_start(out=t[:], in_=src.rearrange("(c p) -> p c", p=P))
            # broadcast rows for the natural-layout V computation
            veff_b = cpool.tile([P, F], FP32)
            beff_b = cpool.tile([P, F], FP32)
            nc.sync.dma_start(out=veff_b[:], in_=veff[None, :].broadcast_to([P, F]))
            nc.sync.dma_start(out=beff_b[:], in_=beff[None, :].broadcast_to([P, F]))

            for b in range(BPC):
                # ---- load x (natural layout) ----
                xts = [xpool.tile([P, F], FP32, tag=f"xt{i}", name=f"xt{i}") for i in range(NT)]
                for i in range(NT):
                    nc.sync.dma_start(out=xts[i][:], in_=x[b, i * P:(i + 1) * P, :])

                # ---- transpose x, produce Q^T / K^T (f32r) ----
                qts = [qpool.tile([P, N], FP32R, tag=f"qt{c}", name=f"qt{c}") for c in range(CT)]
                kts = [kpool.tile([P, N], FP32R, tag=f"kt{c}", name=f"kt{c}") for c in range(CT)]
                TG = min(4, NT)
                for c in range(CT):
                    for g in range(NT // TG):
                        pst = ps_t.tile([P, TG * P], FP32, tag="pst", name="pst")
                        for j in range(TG):
                            i = g * TG + j
                            nc.tensor.transpose(
                                pst[:, j * P:(j + 1) * P],
                                xts[i][:, c * P:(c + 1) * P],
                                ident[:],
                            )
                        sl = slice(g * TG * P, (g + 1) * TG * P)
                        nc.scalar.activation(qts[c][:, sl], pst[:], AF.Identity,
                                             bias=bq_c[:, c:c + 1],
                                             scale=wq_c[:, c:c + 1])
                        nc.vector.tensor_scalar(kts[c][:, sl], pst[:],
                                                wk_c[:, c:c + 1], bk_c[:, c:c + 1],
                                                op0=ALU.mult, op1=ALU.add)

                # ---- Vhat (natural layout, f32r, ones column per head) ----
                vts = [vpool.tile([P, H * dO], FP32R, tag=f"vt{i}", name=f"vt{i}") for i in range(NT)]
                for i in range(NT):
                    v3 = vts[i].rearrange("p (h e) -> p h e", e=dO)
                    x3 = xts[i].rearrange("p (h e) -> p h e", e=d)
                    w3 = veff_b.rearrange("p (h e) -> p h e", e=d)
                    b3 = beff_b.rearrange("p (h e) -> p h e", e=d)
                    nc.vector.tensor_scalar(vts[i][:, d::dO], veff_b[:, 0:H],
                                            0.0, 1.0, op0=ALU.mult, op1=ALU.add)
                    nc.vector.tensor_mul(v3[:, :, 0:d], x3[:], w3[:])
                    nc.vector.tensor_add(v3[:, :, 0:d], v3[:, :, 0:d], b3[:])

                # ---- attention per (head, q-block) ----
                outs = {i: opool.tile([P, F], FP32, tag=f"on{i}", name=f"on{i}")
                        for i in range(NT)}
                for h in range(H):
                    c, off = divmod(h * d, P)
                    for qc in range(QC):
                        po = ps_o.tile([dO, QB], FP32)
                        for g in range(NG):
                            ps = ps_s.tile([P, KG * QB], FP32)
                            for j in range(KG):
                                kc = g * KG + j
                                nc.tensor.matmul(
                                    ps[:, j * QB:(j + 1) * QB],
                                    lhsT=kts[c][off:off + d, kc * P:(kc + 1) * P],
                                    rhs=qts[c][off:off + d, qc * QB:(qc + 1) * QB],
                                    start=True, stop=True,
                                )
                            pt = ptpool.tile([P, KG * QB], FP32R)
                            nc.scalar.activation(pt[:], ps[:], AF.Exp, scale=scale)
                            for j in range(KG):
                                kc = g * KG + j
                                nc.tensor.matmul(
                                    po[:],
                                    lhsT=vts[kc][:, h * dO:(h + 1) * dO],
                                    rhs=pt[:, j * QB:(j + 1) * QB],
                                    start=(kc == 0), stop=(kc == NT - 1),
                                )
                        # ---- drain, transpose to natural, normalize ----
                        ot = otpool.tile([dO, QB], FP32)
                        nc.vector.tensor_copy(out=ot[:], in_=po[:])
                        pn = ps_t.tile([P, TB * dO], FP32, tag="pst")
                        for t in range(TB):
                            nc.tensor.transpose(pn[:, t * dO:(t + 1) * dO],
                                                ot[:, t * P:(t + 1) * P],
                                                ident[0:dO, 0:dO])
                        rz = rzpool.tile([P, TB], FP32)
                        nc.vector.reciprocal(rz[:], pn[:, d::dO])
                        for t in range(TB):
                            qsub = qc * TB + t
                            nc.vector.tensor_scalar_mul(
                                outs[qsub][:, h * d:(h + 1) * d],
                                pn[:, t * dO:t * dO + d],
                                rz[:, t:t + 1],
                            )
                # ---- store ----
                for i in range(NT):
                    nc.sync.dma_start(out=out[b, i * P:(i + 1) * P, :],
                                      in_=outs[i][:])
    nc.compile()
    return nc


_built = {}


def _get_nc(BPC):
    if BPC not in _built:
        _built[BPC] = build(BPC=BPC)
    return _built[BPC]


def kernel(x, wq, bq, wk, bk, wv, bv, wo, bo):
    x = np.ascontiguousarray(np.asarray(x, dtype=np.float32))
    wq, bq, wk, bk, wv, bv, wo, bo = (
        np.asarray(t, dtype=np.float32) for t in (wq, bq, wk, bk, wv, bv, wo, bo))
    Bx = x.shape[0]
    BPC = Bx // N_CORES
    assert BPC * N_CORES == Bx, (Bx, N_CORES)
    veff = wv * wo
    beff = bv * wo + bo
    nc = _get_nc(BPC)
    in_maps = []
    for i in range(N_CORES):
        in_maps.append({
            "x": x[i * BPC:(i + 1) * BPC],
            "wq": wq, "bq": bq, "wk": wk, "bk": bk,
            "veff": veff, "beff": beff,
        })
    res = run_bass_kernel_spmd(nc, in_maps, list(range(N_CORES)))
    return np.concatenate([r["out"] for r in res.results], axis=0)


if __name__ == "__main__":
    rng = np.random.default_rng(1)
    inputs = {
        "x": rng.standard_normal((B, N, F), dtype=np.float32),
        "wq": rng.standard_normal((F,), dtype=np.float32),
        "bq": np.zeros(F, np.float32),
        "wk": rng.standard_normal((F,), dtype=np.float32),
        "bk": np.zeros(F, np.float32),
        "wv": rng.standard_normal((F,), dtype=np.float32),
        "bv": np.zeros(F, np.float32),
        "wo": rng.standard_normal((F,), dtype=np.float32),
        "bo": np.zeros(F, np.float32),
    }
    o = kernel(**inputs)
    print("out", o.shape, o.dtype)
